# revision 21
# baseline (speedup 1.0000x reference)
"""DKVMN forward kernel v5 — software-pipelined scan with engine offload.

Per (student b, m-half h) block i = 2b+h, stages:
  prep(i):  w_flat collapse [DMA], w_bc broadcast [PE matmul + Scalar copy],
            we = w*e [DVE], alpha = 1-we [Scalar], beta = w*a [DVE]
  scan(i):  tensor_tensor_scan over 25 chains of 202 [DVE]  (the bottleneck)
  post(i):  wm = S_shift*w_bc [GpSimd], f_ps += fW1 @ wm_m [PE]

Emission runs prep two blocks ahead of scan so the DVE queue is
we(i+2), beta(i+2), scan(i) back-to-back with alpha(i+2) hidden on Scalar.
Embedding gathers are prefetched per student; p_W gather is deferred to
the tail. Pad columns of the rotating alpha/beta buffers are zeroed once
per buffer version; the scan-chain reset-slot trick is as in v2.
"""
import os
import sys

sys.path.insert(0, "/opt/trn_rl_repo")

import numpy as np
import ml_dtypes

import concourse.bass as bass
import concourse.mybir as mybir
from concourse import bass_utils, tile
from concourse.bacc import Bacc

B, T, NUM_C, D, M = 64, 200, 2000, 128, 50
N_CORES = 8
BC = B // N_CORES
BT = BC * T                  # 1600
NCHUNK = (BT + 127) // 128   # 13
BTP = NCHUNK * 128           # 1664
MH = M // 2                  # 25
CH = T + 2                   # 202: states + pad + reset slot
NBLK = 2 * BC                # 16
PREP_AHEAD = 2
FP32 = mybir.dt.float32
BF16 = mybir.dt.bfloat16
INT32 = mybir.dt.int32

_COMPILED = {}


def _build_nc():
    nc = Bacc("TRN2", target_bir_lowering=False, debug=False, num_devices=N_CORES)

    din = {}
    def dram_in(name, shape, dtype=FP32):
        din[name] = nc.dram_tensor(name, shape, dtype, kind="ExternalInput")
        return din[name]

    dram_in("kidx", [128, NCHUNK], INT32)
    dram_in("vidx", [128, NCHUNK], INT32)
    dram_in("pidx", [128, NCHUNK], INT32)
    dram_in("k_emb", [NUM_C + 1, D])
    dram_in("v_emb", [2 * NUM_C + 1, D])
    dram_in("p_W", [NUM_C, D])
    dram_in("MkT", [D, M])
    dram_in("Mv0T", [D, M])
    dram_in("eWT", [D, D])
    dram_in("aWT", [D, D])
    dram_in("fW1Tb", [D, D], BF16)
    dram_in("fW2Tb", [D, D], BF16)
    dram_in("e_b", [D, 1])
    dram_in("a_b", [D, 1])
    dram_in("f_b", [D, 1])
    dram_in("ident", [D, D])
    dram_in("identb", [D, D], BF16)
    dram_in("ones", [D, 1])
    dram_in("ones_row", [1, D], BF16)
    dram_in("pb_sel", [1, BT])
    out_d = nc.dram_tensor("out", [BC, T], FP32, kind="ExternalOutput")

    AL = mybir.AluOpType
    AF = mybir.ActivationFunctionType

    with tile.TileContext(nc) as tc:
        with (
            tc.tile_pool(name="const", bufs=1) as cpool,
            tc.tile_pool(name="ph1", bufs=1) as ph1,
            tc.tile_pool(name="rows", bufs=3) as rows_p,
            tc.tile_pool(name="wtile", bufs=1) as wt_p,
            tc.tile_pool(name="ab", bufs=3) as ab_p,
            tc.tile_pool(name="sS", bufs=2) as s_p,
            tc.tile_pool(name="wmp", bufs=3) as wm_p,
            tc.tile_pool(name="wbc", bufs=3) as wbc_p,
            tc.tile_pool(name="wfl", bufs=1) as wfl_p,
            tc.tile_pool(name="small", bufs=4) as sm,
            tc.tile_pool(name="psum", bufs=2, space="PSUM") as pp,
            tc.tile_pool(name="psumT", bufs=2, space="PSUM") as ppT,
            tc.tile_pool(name="psumF", bufs=2, space="PSUM") as ppF,
        ):
            def load_const(name, shape, dtype=FP32):
                t = cpool.tile(shape, dtype, tag=name, name=name + "_sb")
                nc.sync.dma_start(t[:], din[name].ap())
                return t

            kidx = load_const("kidx", [128, NCHUNK], INT32)
            vidx = load_const("vidx", [128, NCHUNK], INT32)
            pidx = load_const("pidx", [128, NCHUNK], INT32)
            MkT = load_const("MkT", [D, M])
            Mv0T = load_const("Mv0T", [D, M])
            eWT = load_const("eWT", [D, D])
            aWT = load_const("aWT", [D, D])
            fW1Tb = load_const("fW1Tb", [D, D], BF16)
            fW2Tb = load_const("fW2Tb", [D, D], BF16)
            e_b = load_const("e_b", [D, 1])
            a_b = load_const("a_b", [D, 1])
            f_b = load_const("f_b", [D, 1])
            ident = load_const("ident", [D, D])
            identb = load_const("identb", [D, D], BF16)
            ones = load_const("ones", [D, 1])
            ones_row = load_const("ones_row", [1, D], BF16)
            pb_sel = load_const("pb_sel", [1, BT])

            k_T = ph1.tile([D, BTP], FP32, tag="k_T")
            v_T = ph1.tile([D, BTP], FP32, tag="v_T")
            pw_T = ph1.tile([D, BTP], FP32, tag="pw_T")
            k_Tb = ph1.tile([D, BT], BF16, tag="k_Tb")
            e_sb = ph1.tile([D, BT], BF16, tag="e_sb")
            a_sb = ph1.tile([D, BT], BF16, tag="a_sb")
            w_Tm = ph1.tile([M, BTP], BF16, tag="w_Tm")
            f_sb = ph1.tile([D, BT], FP32, tag="f_sb")

            def gather_chunk(table, idxt, j):
                r = rows_p.tile([128, D], FP32, tag="rows", name="r")
                nc.gpsimd.indirect_dma_start(
                    out=r[:],
                    out_offset=None,
                    in_=din[table].ap(),
                    in_offset=bass.IndirectOffsetOnAxis(ap=idxt[:, j : j + 1], axis=0),
                )
                return r

            def transpose_to(dst, r, j):
                pt = ppT.tile([128, D], FP32, tag="tp", name="pt")
                nc.tensor.transpose(out=pt[:], in_=r[:], identity=ident[:])
                nc.scalar.copy(dst[:, j * 128 : (j + 1) * 128], pt[:])

            kdone = [False] * NCHUNK
            vdone = [False] * NCHUNK

            def need_chunks(hi):
                """Gather + derive w/e/a for chunks <= hi (phase-sorted)."""
                js = [j for j in range(hi + 1) if not kdone[j]]
                for j in js:
                    kdone[j] = True
                    r = gather_chunk("k_emb", kidx, j)
                    transpose_to(k_T, r, j)
                vjs = [j for j in range(hi + 1) if not vdone[j]]
                for j in vjs:
                    vdone[j] = True
                    r = gather_chunk("v_emb", vidx, j)
                    transpose_to(v_T, r, j)
                wts = {}
                for j in js:
                    pw = pp.tile([128, M], FP32, tag="mm", name="pw")
                    nc.tensor.matmul(pw[:], lhsT=k_T[:, j * 128 : (j + 1) * 128],
                                     rhs=MkT[:])
                    nmax = sm.tile([128, 1], FP32, tag="nmax", name="nmax")
                    nc.vector.tensor_reduce(nmax[:], pw[:], axis=mybir.AxisListType.X,
                                            op=AL.max, negate=True)
                    wt = wt_p.tile([128, M], BF16, tag=f"w{j}", name=f"wt{j}")
                    sume = sm.tile([128, 1], FP32, tag="sume", name="sume")
                    nc.scalar.activation(wt[:], pw[:], AF.Exp, bias=nmax[:], scale=1.0,
                                         accum_out=sume[:])
                    rinv = sm.tile([128, 1], FP32, tag="rinv", name="rinv")
                    nc.vector.reciprocal(rinv[:], sume[:])
                    nc.scalar.activation(wt[:], wt[:], AF.Identity, scale=rinv[:])
                    wts[j] = wt
                for j in js:
                    ptw = ppT.tile([M, 128], BF16, tag="tp", name="ptw")
                    nc.tensor.transpose(out=ptw[:], in_=wts[j][:], identity=identb[:])
                    nc.scalar.copy(w_Tm[:, j * 128 : (j + 1) * 128], ptw[:])
                for (wmat, bias, func, dst) in (
                    (eWT, e_b, AF.Sigmoid, e_sb),
                    (aWT, a_b, AF.Tanh, a_sb),
                ):
                    for j in vjs:
                        c0 = j * 128
                        cw = min(128, BT - c0)
                        if cw <= 0:
                            continue
                        pe_ = pp.tile([D, 128], FP32, tag="mm", name="pe_")
                        nc.tensor.matmul(pe_[:, :cw], lhsT=wmat[:],
                                         rhs=v_T[:, c0 : c0 + cw])
                        nc.scalar.activation(dst[:, c0 : c0 + cw], pe_[:, :cw], func,
                                             bias=bias[:], scale=1.0)
                for j in vjs:
                    c0 = j * 128
                    cw = min(128, BT - c0)
                    if cw > 0:
                        nc.scalar.copy(k_Tb[:, c0 : c0 + cw],
                                       k_T[:, c0 : c0 + cw])

            def prep(i):
                b, h = i // 2, i % 2
                m0 = h * MH
                w_flat = wfl_p.tile([1, MH * T], BF16, tag="w_flat", name="w_flat")
                nc.sync.dma_start(
                    w_flat[:].rearrange("p (m t) -> p m t", m=MH),
                    w_Tm[m0 : m0 + MH, b * T : (b + 1) * T],
                )
                w_bc = wbc_p.tile([128, MH * T], BF16, tag="w_bc", name="w_bc")
                for c0 in range(0, MH * T, 500):
                    pb_ps = pp.tile([128, 500], FP32, tag="mm", name="pb_ps")
                    nc.tensor.matmul(pb_ps[:], lhsT=ones_row[:],
                                     rhs=w_flat[:, c0 : c0 + 500])
                    nc.scalar.copy(w_bc[:, c0 : c0 + 500], pb_ps[:])
                w_bc3 = w_bc[:].rearrange("p (m t) -> p m t", m=MH)

                alpha = ab_p.tile([128, MH * CH], BF16, tag="alpha", name="alpha")
                beta = ab_p.tile([128, MH * CH], BF16, tag="beta", name="beta")
                al3 = alpha[:].rearrange("p (m t) -> p m t", m=MH)
                be3 = beta[:].rearrange("p (m t) -> p m t", m=MH)
                e_bc = e_sb[:, b * T : (b + 1) * T].unsqueeze(1).broadcast_to([D, MH, T])
                a_bc = a_sb[:, b * T : (b + 1) * T].unsqueeze(1).broadcast_to([D, MH, T])
                # alpha = 1 - w*e: mult on DVE, affine on Scalar (in place)
                nc.vector.tensor_tensor(al3[:, :, 0:T], w_bc3, e_bc, op=AL.mult)
                nc.scalar.activation(al3[:, :, 0:T], al3[:, :, 0:T],
                                     AF.Identity, bias=ones[:], scale=-1.0)
                nc.vector.tensor_tensor(be3[:, :, 0:T], w_bc3, a_bc, op=AL.mult)
                if i < 3:
                    # pad + reset-slot columns are zeroed once per buffer
                    # version (3-deep rotation); later blocks reuse them
                    nc.vector.memset(al3[:, :, T : T + 2], 0.0)
                    nc.vector.memset(be3[:, :, T : T + 2], 0.0)
                if MH > 1:
                    nc.scalar.copy(
                        be3[:, 0 : MH - 1, T + 1 : T + 2],
                        Mv0T[:, m0 + 1 : m0 + MH].rearrange("p (m o) -> p m o", o=1),
                    )
                return alpha, beta, w_bc3

            def scan_stage(i, alpha, beta):
                b, h = i // 2, i % 2
                m0 = h * MH
                S = s_p.tile([128, 1 + MH * CH], BF16, tag="S", name="S")
                if i < 2:
                    # S version rotation is 2-deep and h alternates, so the
                    # init cell is constant per version
                    nc.scalar.copy(S[:, 0:1], Mv0T[:, m0 : m0 + 1])
                nc.vector.tensor_tensor_scan(
                    S[:, 1 : 1 + MH * CH], alpha[:], beta[:],
                    Mv0T[:, m0 : m0 + 1], op0=AL.mult, op1=AL.add
                )
                return S

            def post(i, S, w_bc3, f_ps):
                b, h = i // 2, i % 2
                wm = wm_p.tile([128, MH * T], BF16, tag="wm", name="wm")
                wm3 = wm[:].rearrange("p (m t) -> p m t", m=MH)
                s_shift = S[:, 0 : MH * CH].rearrange("p (m t) -> p m t", m=MH)[:, :, 0:T]
                nc.vector.tensor_tensor(wm3, s_shift, w_bc3, op=AL.mult)
                for m in range(MH):
                    nc.tensor.matmul(
                        f_ps[:], lhsT=fW1Tb[:], rhs=wm3[:, m, :],
                        start=(h == 0 and m == 0), stop=False,
                    )
                if h == 1:
                    nc.tensor.matmul(f_ps[:], lhsT=fW2Tb[:],
                                     rhs=k_Tb[:, b * T : (b + 1) * T],
                                     start=False, stop=True)
                    nc.scalar.activation(f_sb[:, b * T : (b + 1) * T], f_ps[:],
                                         AF.Tanh, bias=f_b[:], scale=1.0)

            def chunks_hi(b):
                return (b * T + T - 1) // 128

            # ---- pipelined emission ----
            need_chunks(chunks_hi(0))
            tiles = {}
            tiles[0] = prep(0)
            if NBLK > 1:
                tiles[1] = prep(1)
            f_ps_cur = None
            for i in range(NBLK):
                b, h = i // 2, i % 2
                if h == 0:
                    f_ps_cur = ppF.tile([D, T], FP32, tag="fps", name="f_ps")
                    if b + 3 < BC:
                        # prefetch gathers/w/e/a two students ahead of prep
                        need_chunks(chunks_hi(b + 3))
                j = i + PREP_AHEAD
                if j < NBLK:
                    nb = j // 2
                    need_chunks(chunks_hi(nb))
                    tiles[j] = prep(j)
                alpha, beta, w_bc3 = tiles[i]
                S = scan_stage(i, alpha, beta)
                post(i, S, w_bc3, f_ps_cur)
                del tiles[i]
                if i == 8:
                    for j2 in range(NCHUNK):
                        r = gather_chunk("p_W", pidx, j2)
                        transpose_to(pw_T, r, j2)

            # ---- pred = sigmoid(sum_d f*pw + pb) ----
            prod = f_sb
            nc.vector.tensor_mul(prod[:], f_sb[:], pw_T[:, :BT])
            pred = ph1.tile([1, BT], FP32, tag="pred")
            for c in range(0, BT, 400):
                ppd = pp.tile([1, 400], FP32, tag="mm", name="ppd")
                nc.tensor.matmul(ppd[:], lhsT=ones[:], rhs=prod[:, c : c + 400])
                nc.scalar.copy(pred[:, c : c + 400], ppd[:])
            nc.vector.tensor_add(pred[:], pred[:], pb_sel[:])
            nc.scalar.activation(pred[:], pred[:], AF.Sigmoid)
            nc.sync.dma_start(out_d.ap().rearrange("b t -> (b t)").unsqueeze(0), pred[:])

    nc.compile()
    return nc


def _prep_inputs(skill, answer, k_emb, v_emb, Mk, Mv0, f_W, f_b, p_W, p_b,
                 e_W, e_b, a_W, a_b):
    skill = np.asarray(skill)
    answer = np.asarray(answer)
    answer_x = np.where(answer == 2, 1, answer)
    x = (skill + NUM_C * answer_x).astype(np.int64)
    nxt = np.concatenate([skill[:, 1:], np.zeros((B, 1), skill.dtype)], axis=1)
    pidx_full = np.minimum(nxt, NUM_C - 1).astype(np.int64)

    def idx_tiles(a):
        flat = np.zeros(BTP, np.int32)
        flat[:BT] = a.reshape(-1).astype(np.int32)
        return np.ascontiguousarray(flat.reshape(NCHUNK, 128).T)

    bf = ml_dtypes.bfloat16
    common = {
        "k_emb": np.ascontiguousarray(k_emb, np.float32),
        "v_emb": np.ascontiguousarray(v_emb, np.float32),
        "p_W": np.ascontiguousarray(p_W, np.float32),
        "MkT": np.ascontiguousarray(Mk.T, np.float32),
        "Mv0T": np.ascontiguousarray(Mv0.T, np.float32),
        "eWT": np.ascontiguousarray(e_W.T, np.float32),
        "aWT": np.ascontiguousarray(a_W.T, np.float32),
        "fW1Tb": np.ascontiguousarray(f_W[:, :D].T).astype(bf),
        "fW2Tb": np.ascontiguousarray(f_W[:, D:].T).astype(bf),
        "e_b": np.ascontiguousarray(e_b.reshape(D, 1), np.float32),
        "a_b": np.ascontiguousarray(a_b.reshape(D, 1), np.float32),
        "f_b": np.ascontiguousarray(f_b.reshape(D, 1), np.float32),
        "ident": np.eye(D, dtype=np.float32),
        "identb": np.eye(D, dtype=np.float32).astype(bf),
        "ones": np.ones((D, 1), np.float32),
        "ones_row": np.ones((1, D)).astype(ml_dtypes.bfloat16),
    }
    in_maps = []
    for c in range(N_CORES):
        sl = slice(c * BC, (c + 1) * BC)
        m = dict(common)
        m["kidx"] = idx_tiles(skill[sl])
        m["vidx"] = idx_tiles(x[sl])
        m["pidx"] = idx_tiles(pidx_full[sl])
        m["pb_sel"] = np.ascontiguousarray(
            np.asarray(p_b, np.float32)[pidx_full[sl]].reshape(1, BT))
        in_maps.append(m)
    return in_maps


def _install_ntff_hook_shim():
    """Provide antenv.axon_hooks with a ctypes NTFF profile hook when the
    container's antenv package lacks it (needed only for trace=True)."""
    import types
    import ctypes
    import contextlib

    try:
        from antenv.axon_hooks import get_axon_ntff_profile_hook  # noqa: F401
        return
    except ImportError:
        pass

    so_path = "/opt/axon/libaxon_pjrt.so"
    hook = None
    if os.path.exists(so_path):
        lib = ctypes.CDLL(so_path)
        if hasattr(lib, "axon_start_nrt_profile"):
            lib.axon_start_nrt_profile.argtypes = [
                ctypes.POINTER(ctypes.c_int64), ctypes.c_size_t]
            lib.axon_start_nrt_profile.restype = ctypes.c_int64
            lib.axon_stop_nrt_profile.argtypes = [ctypes.c_char_p]
            lib.axon_stop_nrt_profile.restype = ctypes.c_int64

            @contextlib.contextmanager
            def _hook(output_dir, device_ids):
                import jax
                jax.devices()
                if device_ids:
                    ids = (ctypes.c_int64 * len(device_ids))(*device_ids)
                    rc = lib.axon_start_nrt_profile(ids, len(device_ids))
                else:
                    rc = lib.axon_start_nrt_profile(None, 0)
                if rc != 0:
                    raise RuntimeError(f"axon_start_nrt_profile rc={rc}")
                try:
                    yield
                finally:
                    n = lib.axon_stop_nrt_profile(str(output_dir).encode())
                    print(f"profile: {n} file(s) written to {output_dir}",
                          file=sys.stderr)

            hook = _hook

    mod = types.ModuleType("antenv.axon_hooks")
    mod._hook = hook
    mod.get_axon_ntff_profile_hook = lambda: mod._hook
    mod.set_axon_ntff_profile_hook = lambda h: setattr(mod, "_hook", h)
    import antenv
    antenv.axon_hooks = mod
    sys.modules["antenv.axon_hooks"] = mod




def kernel(**inputs):
    if "nc" not in _COMPILED:
        _COMPILED["nc"] = _build_nc()
    nc = _COMPILED["nc"]
    in_maps = _prep_inputs(**inputs)
    trace = bool(int(os.environ.get("KERNEL_TRACE", "0")))
    if trace:
        _install_ntff_hook_shim()
    res = bass_utils.run_bass_kernel_spmd(
        nc, in_maps, core_ids=list(range(N_CORES)), trace=trace
    )
    _COMPILED["last_result"] = res
    out = np.concatenate([res.results[c]["out"][:, : T - 1] for c in range(N_CORES)], axis=0)
    return out.astype(np.float32)


# revision 22
# speedup vs baseline: 1.0768x; 1.0768x over previous
"""DKVMN forward kernel v5 — software-pipelined scan with engine offload.

Per (student b, m-half h) block i = 2b+h, stages:
  prep(i):  w_flat collapse [DMA], w_bc broadcast [PE matmul + Scalar copy],
            we = w*e [DVE], alpha = 1-we [Scalar], beta = w*a [DVE]
  scan(i):  tensor_tensor_scan over 25 chains of 202 [DVE]  (the bottleneck)
  post(i):  wm = S_shift*w_bc [GpSimd], f_ps += fW1 @ wm_m [PE]

Emission runs prep two blocks ahead of scan so the DVE queue is
we(i+2), beta(i+2), scan(i) back-to-back with alpha(i+2) hidden on Scalar.
Embedding gathers are prefetched per student; p_W gather is deferred to
the tail. Pad columns of the rotating alpha/beta buffers are zeroed once
per buffer version; the scan-chain reset-slot trick is as in v2.
"""
import os
import sys

sys.path.insert(0, "/opt/trn_rl_repo")

import numpy as np
import ml_dtypes

import concourse.bass as bass
import concourse.mybir as mybir
from concourse import bass_utils, tile
from concourse.bacc import Bacc

B, T, NUM_C, D, M = 64, 200, 2000, 128, 50
N_CORES = 8
BC = B // N_CORES
BT = BC * T                  # 1600
NCHUNK = (BT + 127) // 128   # 13
BTP = NCHUNK * 128           # 1664
MH = M // 2                  # 25
CH = T + 2                   # 202: states + pad + reset slot
NBLK = 2 * BC                # 16
PREP_AHEAD = 2
FP32 = mybir.dt.float32
BF16 = mybir.dt.bfloat16
INT32 = mybir.dt.int32

_COMPILED = {}


def _build_nc():
    nc = Bacc("TRN2", target_bir_lowering=False, debug=False, num_devices=N_CORES)

    din = {}
    def dram_in(name, shape, dtype=FP32):
        din[name] = nc.dram_tensor(name, shape, dtype, kind="ExternalInput")
        return din[name]

    dram_in("kidx", [128, NCHUNK], INT32)
    dram_in("vidx", [128, NCHUNK], INT32)
    dram_in("pidx", [128, NCHUNK], INT32)
    dram_in("k_emb", [NUM_C + 1, D])
    dram_in("v_emb", [2 * NUM_C + 1, D])
    dram_in("p_W", [NUM_C, D])
    dram_in("MkT", [D, M])
    dram_in("Mv0T", [D, M])
    dram_in("eWT", [D, D])
    dram_in("aWT", [D, D])
    dram_in("fW1Tb", [D, D], BF16)
    dram_in("fW2Tb", [D, D], BF16)
    dram_in("e_b", [D, 1])
    dram_in("a_b", [D, 1])
    dram_in("f_b", [D, 1])
    dram_in("ident", [D, D])
    dram_in("identb", [D, D], BF16)
    dram_in("ones", [D, 1])
    dram_in("ones_row", [1, D], BF16)
    dram_in("pb_sel", [1, BT])
    out_d = nc.dram_tensor("out", [BC, T], FP32, kind="ExternalOutput")

    AL = mybir.AluOpType
    AF = mybir.ActivationFunctionType

    with tile.TileContext(nc) as tc:
        with (
            tc.tile_pool(name="const", bufs=1) as cpool,
            tc.tile_pool(name="ph1", bufs=1) as ph1,
            tc.tile_pool(name="rows", bufs=3) as rows_p,
            tc.tile_pool(name="wtile", bufs=1) as wt_p,
            tc.tile_pool(name="ab", bufs=3) as ab_p,
            tc.tile_pool(name="sS", bufs=2) as s_p,
            tc.tile_pool(name="wmp", bufs=2) as wm_p,
            tc.tile_pool(name="wbc", bufs=3) as wbc_p,
            tc.tile_pool(name="wfl", bufs=2) as wfl_p,
            tc.tile_pool(name="small", bufs=4) as sm,
            tc.tile_pool(name="psum", bufs=2, space="PSUM") as pp,
            tc.tile_pool(name="psumT", bufs=2, space="PSUM") as ppT,
            tc.tile_pool(name="psumF", bufs=2, space="PSUM") as ppF,
        ):
            def load_const(name, shape, dtype=FP32):
                t = cpool.tile(shape, dtype, tag=name, name=name + "_sb")
                nc.sync.dma_start(t[:], din[name].ap())
                return t

            kidx = load_const("kidx", [128, NCHUNK], INT32)
            vidx = load_const("vidx", [128, NCHUNK], INT32)
            pidx = load_const("pidx", [128, NCHUNK], INT32)
            MkT = load_const("MkT", [D, M])
            Mv0T = load_const("Mv0T", [D, M])
            eWT = load_const("eWT", [D, D])
            aWT = load_const("aWT", [D, D])
            fW1Tb = load_const("fW1Tb", [D, D], BF16)
            fW2Tb = load_const("fW2Tb", [D, D], BF16)
            e_b = load_const("e_b", [D, 1])
            a_b = load_const("a_b", [D, 1])
            f_b = load_const("f_b", [D, 1])
            ident = load_const("ident", [D, D])
            identb = load_const("identb", [D, D], BF16)
            ones = load_const("ones", [D, 1])
            ones_row = load_const("ones_row", [1, D], BF16)
            pb_sel = load_const("pb_sel", [1, BT])

            k_T = ph1.tile([D, BTP], FP32, tag="k_T")
            v_T = ph1.tile([D, BTP], FP32, tag="v_T")
            pw_T = ph1.tile([D, BTP], FP32, tag="pw_T")
            k_Tb = ph1.tile([D, BT], BF16, tag="k_Tb")
            e_sb = ph1.tile([D, BT], BF16, tag="e_sb")
            a_sb = ph1.tile([D, BT], BF16, tag="a_sb")
            w_Tm = ph1.tile([M, BTP], BF16, tag="w_Tm")
            f_sb = ph1.tile([D, BT], FP32, tag="f_sb")

            def gather_chunk(table, idxt, j):
                r = rows_p.tile([128, D], FP32, tag="rows", name="r")
                nc.gpsimd.indirect_dma_start(
                    out=r[:],
                    out_offset=None,
                    in_=din[table].ap(),
                    in_offset=bass.IndirectOffsetOnAxis(ap=idxt[:, j : j + 1], axis=0),
                )
                return r

            def transpose_to(dst, r, j):
                pt = ppT.tile([128, D], FP32, tag="tp", name="pt")
                nc.tensor.transpose(out=pt[:], in_=r[:], identity=ident[:])
                nc.scalar.copy(dst[:, j * 128 : (j + 1) * 128], pt[:])

            kdone = [False] * NCHUNK
            vdone = [False] * NCHUNK

            def need_chunks(hi):
                """Gather + derive w/e/a for chunks <= hi (phase-sorted)."""
                js = [j for j in range(hi + 1) if not kdone[j]]
                for j in js:
                    kdone[j] = True
                    r = gather_chunk("k_emb", kidx, j)
                    transpose_to(k_T, r, j)
                vjs = [j for j in range(hi + 1) if not vdone[j]]
                for j in vjs:
                    vdone[j] = True
                    r = gather_chunk("v_emb", vidx, j)
                    transpose_to(v_T, r, j)
                wts = {}
                for j in js:
                    pw = pp.tile([128, M], FP32, tag="mm", name="pw")
                    nc.tensor.matmul(pw[:], lhsT=k_T[:, j * 128 : (j + 1) * 128],
                                     rhs=MkT[:])
                    nmax = sm.tile([128, 1], FP32, tag="nmax", name="nmax")
                    nc.vector.tensor_reduce(nmax[:], pw[:], axis=mybir.AxisListType.X,
                                            op=AL.max, negate=True)
                    wt = wt_p.tile([128, M], BF16, tag=f"w{j}", name=f"wt{j}")
                    sume = sm.tile([128, 1], FP32, tag="sume", name="sume")
                    nc.scalar.activation(wt[:], pw[:], AF.Exp, bias=nmax[:], scale=1.0,
                                         accum_out=sume[:])
                    rinv = sm.tile([128, 1], FP32, tag="rinv", name="rinv")
                    nc.vector.reciprocal(rinv[:], sume[:])
                    nc.scalar.activation(wt[:], wt[:], AF.Identity, scale=rinv[:])
                    wts[j] = wt
                for j in js:
                    ptw = ppT.tile([M, 128], BF16, tag="tp", name="ptw")
                    nc.tensor.transpose(out=ptw[:], in_=wts[j][:], identity=identb[:])
                    nc.scalar.copy(w_Tm[:, j * 128 : (j + 1) * 128], ptw[:])
                for (wmat, bias, func, dst) in (
                    (eWT, e_b, AF.Sigmoid, e_sb),
                    (aWT, a_b, AF.Tanh, a_sb),
                ):
                    for j in vjs:
                        c0 = j * 128
                        cw = min(128, BT - c0)
                        if cw <= 0:
                            continue
                        pe_ = pp.tile([D, 128], FP32, tag="mm", name="pe_")
                        nc.tensor.matmul(pe_[:, :cw], lhsT=wmat[:],
                                         rhs=v_T[:, c0 : c0 + cw])
                        nc.scalar.activation(dst[:, c0 : c0 + cw], pe_[:, :cw], func,
                                             bias=bias[:], scale=1.0)
                for j in vjs:
                    c0 = j * 128
                    cw = min(128, BT - c0)
                    if cw > 0:
                        nc.scalar.copy(k_Tb[:, c0 : c0 + cw],
                                       k_T[:, c0 : c0 + cw])

            def prep(i):
                b, h = i // 2, i % 2
                m0 = h * MH
                w_flat = wfl_p.tile([1, MH * T], BF16, tag="w_flat", name="w_flat")
                nc.sync.dma_start(
                    w_flat[:].rearrange("p (m t) -> p m t", m=MH),
                    w_Tm[m0 : m0 + MH, b * T : (b + 1) * T],
                )
                w_bc = wbc_p.tile([128, MH * T], BF16, tag="w_bc", name="w_bc")
                for c0 in range(0, MH * T, 500):
                    pb_ps = pp.tile([128, 500], FP32, tag="mm", name="pb_ps")
                    nc.tensor.matmul(pb_ps[:], lhsT=ones_row[:],
                                     rhs=w_flat[:, c0 : c0 + 500])
                    nc.scalar.copy(w_bc[:, c0 : c0 + 500], pb_ps[:])
                w_bc3 = w_bc[:].rearrange("p (m t) -> p m t", m=MH)

                alpha = ab_p.tile([128, MH * CH], BF16, tag="alpha", name="alpha")
                beta = ab_p.tile([128, MH * CH], BF16, tag="beta", name="beta")
                al3 = alpha[:].rearrange("p (m t) -> p m t", m=MH)
                be3 = beta[:].rearrange("p (m t) -> p m t", m=MH)
                e_bc = e_sb[:, b * T : (b + 1) * T].unsqueeze(1).broadcast_to([D, MH, T])
                a_bc = a_sb[:, b * T : (b + 1) * T].unsqueeze(1).broadcast_to([D, MH, T])
                # alpha = 1 - w*e: mult on DVE, affine on Scalar (in place)
                nc.vector.tensor_tensor(al3[:, :, 0:T], w_bc3, e_bc, op=AL.mult)
                nc.scalar.activation(al3[:, :, 0:T], al3[:, :, 0:T],
                                     AF.Identity, bias=ones[:], scale=-1.0)
                nc.vector.tensor_tensor(be3[:, :, 0:T], w_bc3, a_bc, op=AL.mult)
                if i < 3:
                    # pad + reset-slot columns are zeroed once per buffer
                    # version (3-deep rotation); later blocks reuse them
                    nc.vector.memset(al3[:, :, T : T + 2], 0.0)
                    nc.vector.memset(be3[:, :, T : T + 2], 0.0)
                if MH > 1:
                    nc.scalar.copy(
                        be3[:, 0 : MH - 1, T + 1 : T + 2],
                        Mv0T[:, m0 + 1 : m0 + MH].rearrange("p (m o) -> p m o", o=1),
                    )
                return alpha, beta, w_bc3

            def scan_stage(i, alpha, beta):
                b, h = i // 2, i % 2
                m0 = h * MH
                S = s_p.tile([128, 1 + MH * CH], BF16, tag="S", name="S")
                if i < 2:
                    # S version rotation is 2-deep and h alternates, so the
                    # init cell is constant per version
                    nc.scalar.copy(S[:, 0:1], Mv0T[:, m0 : m0 + 1])
                nc.vector.tensor_tensor_scan(
                    S[:, 1 : 1 + MH * CH], alpha[:], beta[:],
                    Mv0T[:, m0 : m0 + 1], op0=AL.mult, op1=AL.add
                )
                return S

            def post(i, S, w_bc3, f_ps):
                b, h = i // 2, i % 2
                wm = wm_p.tile([128, MH * T], BF16, tag="wm", name="wm")
                wm3 = wm[:].rearrange("p (m t) -> p m t", m=MH)
                s_shift = S[:, 0 : MH * CH].rearrange("p (m t) -> p m t", m=MH)[:, :, 0:T]
                nc.vector.tensor_tensor(wm3, s_shift, w_bc3, op=AL.mult)
                for m in range(MH):
                    nc.tensor.matmul(
                        f_ps[:], lhsT=fW1Tb[:], rhs=wm3[:, m, :],
                        start=(h == 0 and m == 0), stop=False,
                    )
                if h == 1:
                    nc.tensor.matmul(f_ps[:], lhsT=fW2Tb[:],
                                     rhs=k_Tb[:, b * T : (b + 1) * T],
                                     start=False, stop=True)
                    nc.scalar.activation(f_sb[:, b * T : (b + 1) * T], f_ps[:],
                                         AF.Tanh, bias=f_b[:], scale=1.0)

            def chunks_hi(b):
                return (b * T + T - 1) // 128

            # ---- pipelined emission ----
            need_chunks(chunks_hi(0))
            tiles = {}
            tiles[0] = prep(0)
            if NBLK > 1:
                tiles[1] = prep(1)
            f_ps_cur = None
            for i in range(NBLK):
                b, h = i // 2, i % 2
                if h == 0:
                    f_ps_cur = ppF.tile([D, T], FP32, tag="fps", name="f_ps")
                    if b + 3 < BC:
                        # prefetch gathers/w/e/a two students ahead of prep
                        need_chunks(chunks_hi(b + 3))
                j = i + PREP_AHEAD
                if j < NBLK:
                    nb = j // 2
                    need_chunks(chunks_hi(nb))
                    tiles[j] = prep(j)
                alpha, beta, w_bc3 = tiles[i]
                S = scan_stage(i, alpha, beta)
                post(i, S, w_bc3, f_ps_cur)
                del tiles[i]
                if i == 8:
                    for j2 in range(NCHUNK):
                        r = gather_chunk("p_W", pidx, j2)
                        transpose_to(pw_T, r, j2)

            # ---- pred = sigmoid(sum_d f*pw + pb) ----
            prod = f_sb
            nc.vector.tensor_mul(prod[:], f_sb[:], pw_T[:, :BT])
            pred = ph1.tile([1, BT], FP32, tag="pred")
            for c in range(0, BT, 400):
                ppd = pp.tile([1, 400], FP32, tag="mm", name="ppd")
                nc.tensor.matmul(ppd[:], lhsT=ones[:], rhs=prod[:, c : c + 400])
                nc.scalar.copy(pred[:, c : c + 400], ppd[:])
            nc.vector.tensor_add(pred[:], pred[:], pb_sel[:])
            nc.scalar.activation(pred[:], pred[:], AF.Sigmoid)
            nc.sync.dma_start(out_d.ap().rearrange("b t -> (b t)").unsqueeze(0), pred[:])

    nc.compile()
    return nc


def _prep_inputs(skill, answer, k_emb, v_emb, Mk, Mv0, f_W, f_b, p_W, p_b,
                 e_W, e_b, a_W, a_b):
    skill = np.asarray(skill)
    answer = np.asarray(answer)
    answer_x = np.where(answer == 2, 1, answer)
    x = (skill + NUM_C * answer_x).astype(np.int64)
    nxt = np.concatenate([skill[:, 1:], np.zeros((B, 1), skill.dtype)], axis=1)
    pidx_full = np.minimum(nxt, NUM_C - 1).astype(np.int64)

    def idx_tiles(a):
        flat = np.zeros(BTP, np.int32)
        flat[:BT] = a.reshape(-1).astype(np.int32)
        return np.ascontiguousarray(flat.reshape(NCHUNK, 128).T)

    bf = ml_dtypes.bfloat16
    common = {
        "k_emb": np.ascontiguousarray(k_emb, np.float32),
        "v_emb": np.ascontiguousarray(v_emb, np.float32),
        "p_W": np.ascontiguousarray(p_W, np.float32),
        "MkT": np.ascontiguousarray(Mk.T, np.float32),
        "Mv0T": np.ascontiguousarray(Mv0.T, np.float32),
        "eWT": np.ascontiguousarray(e_W.T, np.float32),
        "aWT": np.ascontiguousarray(a_W.T, np.float32),
        "fW1Tb": np.ascontiguousarray(f_W[:, :D].T).astype(bf),
        "fW2Tb": np.ascontiguousarray(f_W[:, D:].T).astype(bf),
        "e_b": np.ascontiguousarray(e_b.reshape(D, 1), np.float32),
        "a_b": np.ascontiguousarray(a_b.reshape(D, 1), np.float32),
        "f_b": np.ascontiguousarray(f_b.reshape(D, 1), np.float32),
        "ident": np.eye(D, dtype=np.float32),
        "identb": np.eye(D, dtype=np.float32).astype(bf),
        "ones": np.ones((D, 1), np.float32),
        "ones_row": np.ones((1, D)).astype(ml_dtypes.bfloat16),
    }
    in_maps = []
    for c in range(N_CORES):
        sl = slice(c * BC, (c + 1) * BC)
        m = dict(common)
        m["kidx"] = idx_tiles(skill[sl])
        m["vidx"] = idx_tiles(x[sl])
        m["pidx"] = idx_tiles(pidx_full[sl])
        m["pb_sel"] = np.ascontiguousarray(
            np.asarray(p_b, np.float32)[pidx_full[sl]].reshape(1, BT))
        in_maps.append(m)
    return in_maps


def _install_ntff_hook_shim():
    """Provide antenv.axon_hooks with a ctypes NTFF profile hook when the
    container's antenv package lacks it (needed only for trace=True)."""
    import types
    import ctypes
    import contextlib

    try:
        from antenv.axon_hooks import get_axon_ntff_profile_hook  # noqa: F401
        return
    except ImportError:
        pass

    so_path = "/opt/axon/libaxon_pjrt.so"
    hook = None
    if os.path.exists(so_path):
        lib = ctypes.CDLL(so_path)
        if hasattr(lib, "axon_start_nrt_profile"):
            lib.axon_start_nrt_profile.argtypes = [
                ctypes.POINTER(ctypes.c_int64), ctypes.c_size_t]
            lib.axon_start_nrt_profile.restype = ctypes.c_int64
            lib.axon_stop_nrt_profile.argtypes = [ctypes.c_char_p]
            lib.axon_stop_nrt_profile.restype = ctypes.c_int64

            @contextlib.contextmanager
            def _hook(output_dir, device_ids):
                import jax
                jax.devices()
                if device_ids:
                    ids = (ctypes.c_int64 * len(device_ids))(*device_ids)
                    rc = lib.axon_start_nrt_profile(ids, len(device_ids))
                else:
                    rc = lib.axon_start_nrt_profile(None, 0)
                if rc != 0:
                    raise RuntimeError(f"axon_start_nrt_profile rc={rc}")
                try:
                    yield
                finally:
                    n = lib.axon_stop_nrt_profile(str(output_dir).encode())
                    print(f"profile: {n} file(s) written to {output_dir}",
                          file=sys.stderr)

            hook = _hook

    mod = types.ModuleType("antenv.axon_hooks")
    mod._hook = hook
    mod.get_axon_ntff_profile_hook = lambda: mod._hook
    mod.set_axon_ntff_profile_hook = lambda h: setattr(mod, "_hook", h)
    import antenv
    antenv.axon_hooks = mod
    sys.modules["antenv.axon_hooks"] = mod




def kernel(**inputs):
    if "nc" not in _COMPILED:
        _COMPILED["nc"] = _build_nc()
    nc = _COMPILED["nc"]
    in_maps = _prep_inputs(**inputs)
    trace = bool(int(os.environ.get("KERNEL_TRACE", "0")))
    if trace:
        _install_ntff_hook_shim()
    res = bass_utils.run_bass_kernel_spmd(
        nc, in_maps, core_ids=list(range(N_CORES)), trace=trace
    )
    _COMPILED["last_result"] = res
    out = np.concatenate([res.results[c]["out"][:, : T - 1] for c in range(N_CORES)], axis=0)
    return out.astype(np.float32)


# revision 24
# speedup vs baseline: 1.1195x; 1.0397x over previous
"""DKVMN forward kernel v5 — software-pipelined scan with engine offload.

Per (student b, m-half h) block i = 2b+h, stages:
  prep(i):  w_flat collapse [DMA], w_bc broadcast [PE matmul + Scalar copy],
            we = w*e [DVE], alpha = 1-we [Scalar], beta = w*a [DVE]
  scan(i):  tensor_tensor_scan over 25 chains of 202 [DVE]  (the bottleneck)
  post(i):  wm = S_shift*w_bc [GpSimd], f_ps += fW1 @ wm_m [PE]

Emission runs prep two blocks ahead of scan so the DVE queue is
we(i+2), beta(i+2), scan(i) back-to-back with alpha(i+2) hidden on Scalar.
Embedding gathers are prefetched per student; p_W gather is deferred to
the tail. Pad columns of the rotating alpha/beta buffers are zeroed once
per buffer version; the scan-chain reset-slot trick is as in v2.
"""
import os
import sys

sys.path.insert(0, "/opt/trn_rl_repo")

import numpy as np
import ml_dtypes

import concourse.bass as bass
import concourse.mybir as mybir
from concourse import bass_utils, tile
from concourse.bacc import Bacc

B, T, NUM_C, D, M = 64, 200, 2000, 128, 50
N_CORES = 8
BC = B // N_CORES
BT = BC * T                  # 1600
NCHUNK = (BT + 127) // 128   # 13
BTP = NCHUNK * 128           # 1664
MH = M // 2                  # 25
CH = T + 2                   # 202: states + pad + reset slot
NBLK = 2 * BC                # 16
PREP_AHEAD = 2
FP32 = mybir.dt.float32
BF16 = mybir.dt.bfloat16
INT32 = mybir.dt.int32

_COMPILED = {}


def _build_nc():
    nc = Bacc("TRN2", target_bir_lowering=False, debug=False, num_devices=N_CORES)

    din = {}
    def dram_in(name, shape, dtype=FP32):
        din[name] = nc.dram_tensor(name, shape, dtype, kind="ExternalInput")
        return din[name]

    dram_in("kidx", [128, NCHUNK], INT32)
    dram_in("vidx", [128, NCHUNK], INT32)
    dram_in("pidx", [128, NCHUNK], INT32)
    dram_in("k_emb", [NUM_C + 1, D])
    dram_in("v_emb", [2 * NUM_C + 1, D])
    dram_in("p_W", [NUM_C, D])
    dram_in("MkT", [D, M])
    dram_in("Mv0T", [D, M])
    dram_in("eWT", [D, D])
    dram_in("aWT", [D, D])
    dram_in("fW1Tb", [D, D], BF16)
    dram_in("fW2Tb", [D, D], BF16)
    dram_in("e_b", [D, 1])
    dram_in("a_b", [D, 1])
    dram_in("f_b", [D, 1])
    dram_in("ident", [D, D])
    dram_in("identb", [D, D], BF16)
    dram_in("ones", [D, 1])
    dram_in("ones_row", [1, D], BF16)
    dram_in("pb_sel", [1, BT])
    out_d = nc.dram_tensor("out", [BC, T], FP32, kind="ExternalOutput")

    AL = mybir.AluOpType
    AF = mybir.ActivationFunctionType

    with tile.TileContext(nc) as tc:
        with (
            tc.tile_pool(name="const", bufs=1) as cpool,
            tc.tile_pool(name="ph1", bufs=1) as ph1,
            tc.tile_pool(name="rows", bufs=3) as rows_p,
            tc.tile_pool(name="wtile", bufs=1) as wt_p,
            tc.tile_pool(name="ab", bufs=3) as ab_p,
            tc.tile_pool(name="sS", bufs=2) as s_p,
            tc.tile_pool(name="wmp", bufs=2) as wm_p,
            tc.tile_pool(name="wbc", bufs=3) as wbc_p,
            tc.tile_pool(name="wfl", bufs=2) as wfl_p,
            tc.tile_pool(name="small", bufs=4) as sm,
            tc.tile_pool(name="psum", bufs=2, space="PSUM") as pp,
            tc.tile_pool(name="psumT", bufs=2, space="PSUM") as ppT,
            tc.tile_pool(name="psumF", bufs=2, space="PSUM") as ppF,
        ):
            def load_const(name, shape, dtype=FP32):
                t = cpool.tile(shape, dtype, tag=name, name=name + "_sb")
                nc.sync.dma_start(t[:], din[name].ap())
                return t

            kidx = load_const("kidx", [128, NCHUNK], INT32)
            vidx = load_const("vidx", [128, NCHUNK], INT32)
            pidx = load_const("pidx", [128, NCHUNK], INT32)
            MkT = load_const("MkT", [D, M])
            Mv0T = load_const("Mv0T", [D, M])
            eWT = load_const("eWT", [D, D])
            aWT = load_const("aWT", [D, D])
            fW1Tb = load_const("fW1Tb", [D, D], BF16)
            fW2Tb = load_const("fW2Tb", [D, D], BF16)
            e_b = load_const("e_b", [D, 1])
            a_b = load_const("a_b", [D, 1])
            f_b = load_const("f_b", [D, 1])
            ident = load_const("ident", [D, D])
            identb = load_const("identb", [D, D], BF16)
            ones = load_const("ones", [D, 1])
            ones_row = load_const("ones_row", [1, D], BF16)
            pb_sel = load_const("pb_sel", [1, BT])

            k_T = ph1.tile([D, BTP], FP32, tag="k_T")
            v_T = ph1.tile([D, BTP], FP32, tag="v_T")
            pw_T = ph1.tile([D, BTP], FP32, tag="pw_T")
            k_Tb = ph1.tile([D, BT], BF16, tag="k_Tb")
            e_sb = ph1.tile([D, BT], BF16, tag="e_sb")
            a_sb = ph1.tile([D, BT], BF16, tag="a_sb")
            w_Tm = ph1.tile([M, BTP], BF16, tag="w_Tm")
            f_sb = ph1.tile([D, BT], FP32, tag="f_sb")

            def gather_chunk(table, idxt, j):
                r = rows_p.tile([128, D], FP32, tag="rows", name="r")
                nc.gpsimd.indirect_dma_start(
                    out=r[:],
                    out_offset=None,
                    in_=din[table].ap(),
                    in_offset=bass.IndirectOffsetOnAxis(ap=idxt[:, j : j + 1], axis=0),
                )
                return r

            def transpose_to(dst, r, j):
                pt = ppT.tile([128, D], FP32, tag="tp", name="pt")
                nc.tensor.transpose(out=pt[:], in_=r[:], identity=ident[:])
                nc.scalar.copy(dst[:, j * 128 : (j + 1) * 128], pt[:])

            kdone = [False] * NCHUNK
            vdone = [False] * NCHUNK

            def need_chunks(hi):
                """Gather + derive w/e/a for chunks <= hi (phase-sorted)."""
                js = [j for j in range(hi + 1) if not kdone[j]]
                for j in js:
                    kdone[j] = True
                    r = gather_chunk("k_emb", kidx, j)
                    transpose_to(k_T, r, j)
                vjs = [j for j in range(hi + 1) if not vdone[j]]
                for j in vjs:
                    vdone[j] = True
                    r = gather_chunk("v_emb", vidx, j)
                    transpose_to(v_T, r, j)
                wts = {}
                for j in js:
                    pw = pp.tile([128, M], FP32, tag="mm", name="pw")
                    nc.tensor.matmul(pw[:], lhsT=k_T[:, j * 128 : (j + 1) * 128],
                                     rhs=MkT[:])
                    nmax = sm.tile([128, 1], FP32, tag="nmax", name="nmax")
                    nc.vector.tensor_reduce(nmax[:], pw[:], axis=mybir.AxisListType.X,
                                            op=AL.max, negate=True)
                    wt = wt_p.tile([128, M], BF16, tag=f"w{j}", name=f"wt{j}")
                    sume = sm.tile([128, 1], FP32, tag="sume", name="sume")
                    nc.scalar.activation(wt[:], pw[:], AF.Exp, bias=nmax[:], scale=1.0,
                                         accum_out=sume[:])
                    rinv = sm.tile([128, 1], FP32, tag="rinv", name="rinv")
                    nc.vector.reciprocal(rinv[:], sume[:])
                    nc.scalar.activation(wt[:], wt[:], AF.Identity, scale=rinv[:])
                    wts[j] = wt
                for j in js:
                    ptw = ppT.tile([M, 128], BF16, tag="tp", name="ptw")
                    nc.tensor.transpose(out=ptw[:], in_=wts[j][:], identity=identb[:])
                    nc.scalar.copy(w_Tm[:, j * 128 : (j + 1) * 128], ptw[:])
                for (wmat, bias, func, dst) in (
                    (eWT, e_b, AF.Sigmoid, e_sb),
                    (aWT, a_b, AF.Tanh, a_sb),
                ):
                    for j in vjs:
                        c0 = j * 128
                        cw = min(128, BT - c0)
                        if cw <= 0:
                            continue
                        pe_ = pp.tile([D, 128], FP32, tag="mm", name="pe_")
                        nc.tensor.matmul(pe_[:, :cw], lhsT=wmat[:],
                                         rhs=v_T[:, c0 : c0 + cw])
                        nc.scalar.activation(dst[:, c0 : c0 + cw], pe_[:, :cw], func,
                                             bias=bias[:], scale=1.0)
                for j in vjs:
                    c0 = j * 128
                    cw = min(128, BT - c0)
                    if cw > 0:
                        nc.scalar.copy(k_Tb[:, c0 : c0 + cw],
                                       k_T[:, c0 : c0 + cw])

            def prep(i):
                b, h = i // 2, i % 2
                m0 = h * MH
                w_flat = wfl_p.tile([1, MH * T], BF16, tag="w_flat", name="w_flat")
                nc.sync.dma_start(
                    w_flat[:].rearrange("p (m t) -> p m t", m=MH),
                    w_Tm[m0 : m0 + MH, b * T : (b + 1) * T],
                )
                w_bc = wbc_p.tile([128, MH * T], BF16, tag="w_bc", name="w_bc")
                for c0 in range(0, MH * T, 500):
                    pb_ps = pp.tile([128, 500], FP32, tag="mm", name="pb_ps")
                    nc.tensor.matmul(pb_ps[:], lhsT=ones_row[:],
                                     rhs=w_flat[:, c0 : c0 + 500])
                    nc.scalar.copy(w_bc[:, c0 : c0 + 500], pb_ps[:])
                w_bc3 = w_bc[:].rearrange("p (m t) -> p m t", m=MH)

                alpha = ab_p.tile([128, MH * CH], BF16, tag="alpha", name="alpha")
                beta = ab_p.tile([128, MH * CH], BF16, tag="beta", name="beta")
                al3 = alpha[:].rearrange("p (m t) -> p m t", m=MH)
                be3 = beta[:].rearrange("p (m t) -> p m t", m=MH)
                e_bc = e_sb[:, b * T : (b + 1) * T].unsqueeze(1).broadcast_to([D, MH, T])
                a_bc = a_sb[:, b * T : (b + 1) * T].unsqueeze(1).broadcast_to([D, MH, T])
                # alpha = 1 - w*e: mult on DVE, affine on Scalar (in place)
                nc.vector.tensor_tensor(al3[:, :, 0:T], w_bc3, e_bc, op=AL.mult)
                nc.scalar.activation(al3[:, :, 0:T], al3[:, :, 0:T],
                                     AF.Identity, bias=ones[:], scale=-1.0)
                nc.vector.tensor_tensor(be3[:, :, 0:T], w_bc3, a_bc, op=AL.mult)
                if i < 3:
                    # pad + reset-slot columns are zeroed once per buffer
                    # version (3-deep rotation); later blocks reuse them
                    nc.vector.memset(al3[:, :, T : T + 2], 0.0)
                    nc.vector.memset(be3[:, :, T : T + 2], 0.0)
                if MH > 1:
                    nc.scalar.copy(
                        be3[:, 0 : MH - 1, T + 1 : T + 2],
                        Mv0T[:, m0 + 1 : m0 + MH].rearrange("p (m o) -> p m o", o=1),
                    )
                return alpha, beta, w_bc3

            def scan_stage(i, alpha, beta):
                b, h = i // 2, i % 2
                m0 = h * MH
                S = s_p.tile([128, 1 + MH * CH], BF16, tag="S", name="S")
                if i < 2:
                    # S version rotation is 2-deep and h alternates, so the
                    # init cell is constant per version
                    nc.scalar.copy(S[:, 0:1], Mv0T[:, m0 : m0 + 1])
                nc.vector.tensor_tensor_scan(
                    S[:, 1 : 1 + MH * CH], alpha[:], beta[:],
                    Mv0T[:, m0 : m0 + 1], op0=AL.mult, op1=AL.add
                )
                return S

            def post(i, S, w_bc3, f_ps):
                b, h = i // 2, i % 2
                wm = wm_p.tile([128, MH * T], BF16, tag="wm", name="wm")
                wm3 = wm[:].rearrange("p (m t) -> p m t", m=MH)
                s_shift = S[:, 0 : MH * CH].rearrange("p (m t) -> p m t", m=MH)[:, :, 0:T]
                nc.vector.tensor_tensor(wm3, s_shift, w_bc3, op=AL.mult)
                for m in range(MH):
                    nc.tensor.matmul(
                        f_ps[:], lhsT=fW1Tb[:], rhs=wm3[:, m, :],
                        start=(h == 0 and m == 0), stop=False,
                    )
                if h == 1:
                    nc.tensor.matmul(f_ps[:], lhsT=fW2Tb[:],
                                     rhs=k_Tb[:, b * T : (b + 1) * T],
                                     start=False, stop=True)
                    nc.scalar.activation(f_sb[:, b * T : (b + 1) * T], f_ps[:],
                                         AF.Tanh, bias=f_b[:], scale=1.0)

            def chunks_hi(b):
                return (b * T + T - 1) // 128

            # ---- pipelined emission ----
            need_chunks(chunks_hi(min(2, BC - 1)))
            tiles = {}
            tiles[0] = prep(0)
            if NBLK > 1:
                tiles[1] = prep(1)
            f_ps_cur = None
            for i in range(NBLK):
                b, h = i // 2, i % 2
                if h == 0:
                    f_ps_cur = ppF.tile([D, T], FP32, tag="fps", name="f_ps")
                    if b + 3 < BC:
                        # prefetch gathers/w/e/a two students ahead of prep
                        need_chunks(chunks_hi(b + 3))
                j = i + PREP_AHEAD
                if j < NBLK:
                    nb = j // 2
                    need_chunks(chunks_hi(nb))
                    tiles[j] = prep(j)
                alpha, beta, w_bc3 = tiles[i]
                S = scan_stage(i, alpha, beta)
                post(i, S, w_bc3, f_ps_cur)
                del tiles[i]
                if 10 <= i <= 12:
                    for j2 in range((i - 10) * 5, min(NCHUNK, (i - 9) * 5)):
                        r = gather_chunk("p_W", pidx, j2)
                        transpose_to(pw_T, r, j2)

            # ---- pred = sigmoid(sum_d f*pw + pb) ----
            prod = f_sb
            nc.vector.tensor_mul(prod[:], f_sb[:], pw_T[:, :BT])
            pred = ph1.tile([1, BT], FP32, tag="pred")
            for c in range(0, BT, 400):
                ppd = pp.tile([1, 400], FP32, tag="mm", name="ppd")
                nc.tensor.matmul(ppd[:], lhsT=ones[:], rhs=prod[:, c : c + 400])
                nc.scalar.copy(pred[:, c : c + 400], ppd[:])
            nc.vector.tensor_add(pred[:], pred[:], pb_sel[:])
            nc.scalar.activation(pred[:], pred[:], AF.Sigmoid)
            nc.sync.dma_start(out_d.ap().rearrange("b t -> (b t)").unsqueeze(0), pred[:])

    nc.compile()
    return nc


def _prep_inputs(skill, answer, k_emb, v_emb, Mk, Mv0, f_W, f_b, p_W, p_b,
                 e_W, e_b, a_W, a_b):
    skill = np.asarray(skill)
    answer = np.asarray(answer)
    answer_x = np.where(answer == 2, 1, answer)
    x = (skill + NUM_C * answer_x).astype(np.int64)
    nxt = np.concatenate([skill[:, 1:], np.zeros((B, 1), skill.dtype)], axis=1)
    pidx_full = np.minimum(nxt, NUM_C - 1).astype(np.int64)

    def idx_tiles(a):
        flat = np.zeros(BTP, np.int32)
        flat[:BT] = a.reshape(-1).astype(np.int32)
        return np.ascontiguousarray(flat.reshape(NCHUNK, 128).T)

    bf = ml_dtypes.bfloat16
    common = {
        "k_emb": np.ascontiguousarray(k_emb, np.float32),
        "v_emb": np.ascontiguousarray(v_emb, np.float32),
        "p_W": np.ascontiguousarray(p_W, np.float32),
        "MkT": np.ascontiguousarray(Mk.T, np.float32),
        "Mv0T": np.ascontiguousarray(Mv0.T, np.float32),
        "eWT": np.ascontiguousarray(e_W.T, np.float32),
        "aWT": np.ascontiguousarray(a_W.T, np.float32),
        "fW1Tb": np.ascontiguousarray(f_W[:, :D].T).astype(bf),
        "fW2Tb": np.ascontiguousarray(f_W[:, D:].T).astype(bf),
        "e_b": np.ascontiguousarray(e_b.reshape(D, 1), np.float32),
        "a_b": np.ascontiguousarray(a_b.reshape(D, 1), np.float32),
        "f_b": np.ascontiguousarray(f_b.reshape(D, 1), np.float32),
        "ident": np.eye(D, dtype=np.float32),
        "identb": np.eye(D, dtype=np.float32).astype(bf),
        "ones": np.ones((D, 1), np.float32),
        "ones_row": np.ones((1, D)).astype(ml_dtypes.bfloat16),
    }
    in_maps = []
    for c in range(N_CORES):
        sl = slice(c * BC, (c + 1) * BC)
        m = dict(common)
        m["kidx"] = idx_tiles(skill[sl])
        m["vidx"] = idx_tiles(x[sl])
        m["pidx"] = idx_tiles(pidx_full[sl])
        m["pb_sel"] = np.ascontiguousarray(
            np.asarray(p_b, np.float32)[pidx_full[sl]].reshape(1, BT))
        in_maps.append(m)
    return in_maps


def _install_ntff_hook_shim():
    """Provide antenv.axon_hooks with a ctypes NTFF profile hook when the
    container's antenv package lacks it (needed only for trace=True)."""
    import types
    import ctypes
    import contextlib

    try:
        from antenv.axon_hooks import get_axon_ntff_profile_hook  # noqa: F401
        return
    except ImportError:
        pass

    so_path = "/opt/axon/libaxon_pjrt.so"
    hook = None
    if os.path.exists(so_path):
        lib = ctypes.CDLL(so_path)
        if hasattr(lib, "axon_start_nrt_profile"):
            lib.axon_start_nrt_profile.argtypes = [
                ctypes.POINTER(ctypes.c_int64), ctypes.c_size_t]
            lib.axon_start_nrt_profile.restype = ctypes.c_int64
            lib.axon_stop_nrt_profile.argtypes = [ctypes.c_char_p]
            lib.axon_stop_nrt_profile.restype = ctypes.c_int64

            @contextlib.contextmanager
            def _hook(output_dir, device_ids):
                import jax
                jax.devices()
                if device_ids:
                    ids = (ctypes.c_int64 * len(device_ids))(*device_ids)
                    rc = lib.axon_start_nrt_profile(ids, len(device_ids))
                else:
                    rc = lib.axon_start_nrt_profile(None, 0)
                if rc != 0:
                    raise RuntimeError(f"axon_start_nrt_profile rc={rc}")
                try:
                    yield
                finally:
                    n = lib.axon_stop_nrt_profile(str(output_dir).encode())
                    print(f"profile: {n} file(s) written to {output_dir}",
                          file=sys.stderr)

            hook = _hook

    mod = types.ModuleType("antenv.axon_hooks")
    mod._hook = hook
    mod.get_axon_ntff_profile_hook = lambda: mod._hook
    mod.set_axon_ntff_profile_hook = lambda h: setattr(mod, "_hook", h)
    import antenv
    antenv.axon_hooks = mod
    sys.modules["antenv.axon_hooks"] = mod




def kernel(**inputs):
    if "nc" not in _COMPILED:
        _COMPILED["nc"] = _build_nc()
    nc = _COMPILED["nc"]
    in_maps = _prep_inputs(**inputs)
    trace = bool(int(os.environ.get("KERNEL_TRACE", "0")))
    if trace:
        _install_ntff_hook_shim()
    res = bass_utils.run_bass_kernel_spmd(
        nc, in_maps, core_ids=list(range(N_CORES)), trace=trace
    )
    _COMPILED["last_result"] = res
    out = np.concatenate([res.results[c]["out"][:, : T - 1] for c in range(N_CORES)], axis=0)
    return out.astype(np.float32)
